# revision 1
# baseline (speedup 1.0000x reference)
"""HGNN encoder (2-layer hypergraph message passing) as an 8-core TRN2 Bass kernel.

Final: bf16/fp8 data path (fp8-e4m3 incidence matrices, bf16 embeddings
and messages), split user/item bf16 AllReduces, phase-D transposed-output
matmuls with paired-chunk [128,1024] incidence loads, dual-DGE streaming
(uhT/ihT on SP, fhT on Activation). KREPS (env) repeats the body for
device-bound timing; kernel() always uses the single-rep build. Strategy (1D node partition, K-sharded G-matmuls):
  - Each core owns a contiguous shard of user nodes (U/8) and item nodes (I/8).
  - All big operands (incidence matrices, embeddings) are cast to bf16 on the
    host: halves HBM traffic (this is a memory-bound problem) and runs the
    PE at 1 cycle/row instead of fp32's 4.
  - Phase A: user_hyper@user_emb / item_hyper@item_emb contract over nodes;
    each core multiplies its node-columns of the (host-pre-transposed)
    incidence slices against its node shard of the embeddings -> partial
    [64, G] messages in PSUM; one packed [128, G] bf16 AllReduce yields the
    full messages everywhere.
  - Phase B (attention/update, tiny) runs redundantly per core in a
    transposed [64, G] layout; all matmuls layout-natural.
  - Phase D: full_hyper@msg contracts over G with TRANSPOSED output
    [64, nodes]: 16 PSUM-accumulated matmuls of 512 free-dim rows per
    512-node chunk (instead of many tiny [<=128, 64] matmuls), then PE
    transposes back to node-major for the residents/outputs.
"""

import os
import numpy as np

U, I, G, D = 30000, 60000, 2000, 64
L = 2

NCORES = 8
UC, IC = U // NCORES, I // NCORES
GPAD = 2048


def _ktiles(n, step=128):
    return [(s, min(step, n - s)) for s in range(0, n, step)]


MSCS = (32.0, float(2 ** 27))  # per-layer msgP fp8 scale: |msg| maxes 887 and 1.26e10
GCH = _ktiles(G, 512)     # 4 free-dim chunks for matmul N<=512
GTL = _ktiles(G, 128)     # 16 partition tiles


def _build():
    import concourse.bacc as bacc
    import concourse.mybir as mybir
    import concourse.tile as tile
    from concourse import masks

    reps = int(os.environ.get("KREPS", "1"))
    f32 = mybir.dt.float32
    bf16 = mybir.dt.bfloat16
    fp8 = mybir.dt.float8e4
    nc = bacc.Bacc("TRN2", target_bir_lowering=False, debug=False,
                   num_devices=NCORES)

    def din(name, shape, dt=bf16):
        return nc.dram_tensor(name, shape, dt, kind="ExternalInput").ap()

    def dout(name, shape):
        return nc.dram_tensor(name, shape, mybir.dt.float32,
                              kind="ExternalOutput").ap()

    ue = din("ue", [UC, D])
    ie = din("ie", [IC, D])
    uhT = din("uhT", [UC, G], fp8)
    ihT = din("ihT", [IC, G], fp8)
    fhTu = din("fhTu", [GPAD, UC], fp8)
    fhTi = din("fhTi", [GPAD, IC], fp8)
    gT = din("gT", [D, G])
    qc_w1 = din("qc_w1", [L, D, D])
    qc_b1 = din("qc_b1", [L, D], f32)
    qc_w2 = din("qc_w2", [L, D, 1])
    user_w = din("user_w", [L, 2 * D, D])
    user_b = din("user_b", [L, D], f32)
    item_w = din("item_w", [L, 2 * D, D])
    item_b = din("item_b", [L, D], f32)

    final_u = dout("final_u", [UC, D])
    final_i = dout("final_i", [IC, D])
    final_he = dout("final_he", [G, D])

    ukt = _ktiles(UC)   # 30 k-tiles for the user shard
    ikt = _ktiles(IC)   # 59 k-tiles for the item shard

    with tile.TileContext(nc) as tc:
        with (
            tc.tile_pool(name="const", bufs=1) as cpool,
            tc.tile_pool(name="resid", bufs=1) as rpool,
            tc.tile_pool(name="rhsA", bufs=6) as apool,
            tc.tile_pool(name="fhD", bufs=20) as dpool,
            tc.tile_pool(name="pb", bufs=2) as bpool,
            tc.tile_pool(name="outp", bufs=4) as opool,
            tc.tile_pool(name="psA", bufs=1, space="PSUM") as ps_a,
            tc.tile_pool(name="psD", bufs=2, space="PSUM") as ps_d,
            tc.tile_pool(name="psB", bufs=2, space="PSUM") as ps_bc,
            tc.tile_pool(name="dram", bufs=1, space="DRAM") as drpool,
        ):
            # ---- constants / weights -------------------------------------
            identB = cpool.tile([128, 128], bf16, tag="identB", name="identB")
            masks.make_identity(nc, identB[:])
            identF = cpool.tile([64, 64], f32, tag="identF", name="identF")
            masks.make_identity(nc, identF[:])
            ones1 = cpool.tile([1, D], bf16, tag="ones1", name="ones1")
            nc.vector.memset(ones1[:], 1.0)
            zsc = cpool.tile([128, 512], bf16, tag="zsc", name="zsc")
            nc.vector.memset(zsc[:], 0.0)

            gT_s = cpool.tile([D, GPAD], bf16, tag="gTs", name="gT_s")
            nc.scalar.dma_start(gT_s[:, :G], gT[:, :])

            w1_s, w2_s, b1_s, uw_s, iw_s, wsum_s, bsum_s = \
                [], [], [], [], [], [], []
            for l in range(L):
                w1 = cpool.tile([D, D], bf16, tag=f"w1_{l}", name=f"w1_{l}")
                nc.scalar.dma_start(w1[:], qc_w1[l])
                w1_s.append(w1)
                w2 = cpool.tile([D, 1], bf16, tag=f"w2_{l}", name=f"w2_{l}")
                nc.scalar.dma_start(w2[:], qc_w2[l])
                w2n = cpool.tile([D, 1], bf16, tag=f"w2n_{l}", name=f"w2n_{l}")
                nc.scalar.mul(w2n[:], w2[:], -1.0)
                w2_s.append((w2, w2n))
                b1 = cpool.tile([D, 1], f32, tag=f"b1_{l}", name=f"b1_{l}")
                nc.scalar.dma_start(b1[:], qc_b1[l].unsqueeze(1))
                b1_s.append(b1)
                uw0 = cpool.tile([D, D], bf16, tag=f"uw0_{l}", name=f"uw0_{l}")
                nc.scalar.dma_start(uw0[:], user_w[l][0:D, :])
                uw1 = cpool.tile([D, D], bf16, tag=f"uw1_{l}", name=f"uw1_{l}")
                nc.scalar.dma_start(uw1[:], user_w[l][D:2 * D, :])
                uw_s.append(uw0)
                iw0 = cpool.tile([D, D], bf16, tag=f"iw0_{l}", name=f"iw0_{l}")
                nc.scalar.dma_start(iw0[:], item_w[l][0:D, :])
                iw1 = cpool.tile([D, D], bf16, tag=f"iw1_{l}", name=f"iw1_{l}")
                nc.scalar.dma_start(iw1[:], item_w[l][D:2 * D, :])
                iw_s.append(iw0)
                ws = cpool.tile([D, D], bf16, tag=f"ws_{l}", name=f"ws_{l}")
                nc.vector.tensor_add(ws[:], uw1[:], iw1[:])
                wsum_s.append(ws)
                ub = cpool.tile([D, 1], f32, tag=f"ub_{l}", name=f"ub_{l}")
                nc.scalar.dma_start(ub[:], user_b[l].unsqueeze(1))
                ib = cpool.tile([D, 1], f32, tag=f"ib_{l}", name=f"ib_{l}")
                nc.scalar.dma_start(ib[:], item_b[l].unsqueeze(1))
                bs = cpool.tile([D, 1], f32, tag=f"bs_{l}", name=f"bs_{l}")
                nc.vector.tensor_add(bs[:], ub[:], ib[:])
                bsum_s.append(bs)

            # ---- residents ----------------------------------------------
            ue_res = rpool.tile([128, len(ukt) * D], bf16, tag="ue_res",
                                name="ue_res")
            ie_res = rpool.tile([128, len(ikt) * D], bf16, tag="ie_res",
                                name="ie_res")
            ui_u = rpool.tile([128, len(ukt) * D], bf16, tag="ui_u",
                              name="ui_u")
            ui_i = rpool.tile([128, len(ikt) * D], bf16, tag="ui_i",
                              name="ui_i")
            acc_u = rpool.tile([128, len(ukt) * D], f32, tag="acc_u",
                               name="acc_u")
            acc_i = rpool.tile([128, len(ikt) * D], f32, tag="acc_i",
                               name="acc_i")
            he_acc = rpool.tile([D, GPAD], f32, tag="he_acc", name="he_acc")
            for t, (k0, kk) in enumerate(ukt):
                nc.sync.dma_start(ue_res[:kk, t * D:(t + 1) * D],
                                  ue[k0:k0 + kk, :])
            for t, (k0, kk) in enumerate(ikt):
                nc.scalar.dma_start(ie_res[:kk, t * D:(t + 1) * D],
                                    ie[k0:k0 + kk, :])
            nc.vector.tensor_copy(he_acc[:, :G], gT_s[:, :G])

            for rep_l in range(reps * L):
                rep, l = divmod(rep_l, L)
                # ==== Phase A: partial messages, K-sharded over nodes ====
                # user and item partials AllReduce SEPARATELY so the user
                # collective flies while the item matmuls still run.
                skip_cc = os.environ.get("KSKIP_CC") == "1"
                msgTs = []
                for part, (hyT, kt, emb0, emb1) in enumerate([
                        (uhT, ukt, ue_res, ui_u), (ihT, ikt, ie_res, ui_i)]):
                    lhs_res = emb0 if l == 0 else emb1
                    ps_msg = ps_a.tile([64, GPAD], f32, tag="msgps",
                                       name=f"msgps_{rep_l}_{part}")
                    nkt = len(kt)
                    for t, (k0, kk) in enumerate(kt):
                        rt = apool.tile([128, GPAD], fp8, tag="rhsA",
                                        name=f"rhsA_{rep_l}_{part}_{t}")
                        nc.sync.dma_start(rt[:kk, :G], hyT[k0:k0 + kk, :])
                        for (g0, gw) in GCH:
                            nc.tensor.matmul(
                                ps_msg[:, g0:g0 + gw],
                                lhsT=lhs_res[:kk, t * D:(t + 1) * D],
                                rhs=rt[:kk, g0:g0 + gw],
                                start=(t == 0), stop=(t == nkt - 1))
                    pdrain = bpool.tile([64, G], bf16, tag=f"pdrain{part}",
                                        bufs=1, name=f"pdrain_{rep_l}_{part}")
                    nc.vector.tensor_copy(pdrain[:, :], ps_msg[:, :G])
                    # AllReduce in two G-halves: phase B's early chunks gate
                    # on the first half only, cutting the exposed AR latency
                    msgT = bpool.tile([64, GPAD], bf16, tag=f"msgT{part}",
                                      bufs=2, name=f"msgT_{rep_l}_{part}")
                    for hf, (h0, hw) in enumerate(((0, 1024), (1024, G - 1024))):
                        cc_in = drpool.tile(
                            [64, hw], bf16, tag=f"cc_in_{rep_l}_{part}_{hf}",
                            name=f"cc_in_{rep_l}_{part}_{hf}")
                        nc.sync.dma_start(cc_in[:, :],
                                          pdrain[:, h0:h0 + hw])
                        if skip_cc:
                            cc_out = cc_in
                        else:
                            cc_out = drpool.tile(
                                [64, hw], bf16,
                                tag=f"cc_out_{rep_l}_{part}_{hf}",
                                addr_space="Shared",
                                name=f"cc_out_{rep_l}_{part}_{hf}")
                            nc.gpsimd.collective_compute(
                                "AllReduce", mybir.AluOpType.add,
                                ins=[cc_in.opt()], outs=[cc_out.opt()],
                                replica_groups=[list(range(NCORES))])
                        nc.sync.dma_start(msgT[:, h0:h0 + hw], cc_out[:, :])
                    msgTs.append(msgT)
                msgT_u, msgT_i = msgTs

                # ==== Phase B: attention + node-update weights (full G) ====
                msgNT = bpool.tile([64, GPAD], bf16, tag="msgNT", bufs=2,
                                   name=f"msgNT_{rep_l}")
                nc.vector.memset(msgNT[:, G:GPAD], 0.0)
                for ci, (g0, gw) in enumerate(GCH):
                    sl = slice(g0, g0 + gw)
                    um = msgT_u[:, sl]
                    im = msgT_i[:, sl]
                    # h = tanh(msg @ W1 + b1), transposed layout
                    hu_ps = ps_bc.tile([64, 512], f32, tag="pb",
                                       name=f"hu_{rep_l}_{ci}")
                    nc.tensor.matmul(hu_ps[:, :gw], lhsT=w1_s[l][:], rhs=um,
                                     start=True, stop=True)
                    hu = bpool.tile([64, 512], bf16, tag="hu",
                                    name=f"hus_{rep_l}_{ci}")
                    nc.scalar.activation(hu[:, :gw], hu_ps[:, :gw],
                                         mybir.ActivationFunctionType.Tanh,
                                         bias=b1_s[l][:])
                    hi_ps = ps_bc.tile([64, 512], f32, tag="pb",
                                       name=f"hi_{rep_l}_{ci}")
                    nc.tensor.matmul(hi_ps[:, :gw], lhsT=w1_s[l][:], rhs=im,
                                     start=True, stop=True)
                    hi = bpool.tile([64, 512], bf16, tag="hi",
                                    name=f"his_{rep_l}_{ci}")
                    nc.scalar.activation(hi[:, :gw], hi_ps[:, :gw],
                                         mybir.ActivationFunctionType.Tanh,
                                         bias=b1_s[l][:])
                    # attention logit diff a_u - a_i in one bank;
                    # softmax over 2 == sigmoid(+-diff)
                    ad_ps = ps_bc.tile([1, 512], f32, tag="pb",
                                       name=f"ad_{rep_l}_{ci}")
                    nc.tensor.matmul(ad_ps[:, :gw], lhsT=w2_s[l][0][:],
                                     rhs=hu[:, :gw], start=True, stop=False)
                    nc.tensor.matmul(ad_ps[:, :gw], lhsT=w2_s[l][1][:],
                                     rhs=hi[:, :gw], start=False, stop=True)
                    wu = bpool.tile([1, 512], bf16, tag="wud", bufs=4,
                                    name=f"wu_{rep_l}_{ci}")
                    nc.scalar.activation(wu[:, :gw], ad_ps[:, :gw],
                                         mybir.ActivationFunctionType.Sigmoid)
                    wi = bpool.tile([1, 512], bf16, tag="wud", bufs=4,
                                    name=f"wi_{rep_l}_{ci}")
                    nc.scalar.activation(wi[:, :gw], ad_ps[:, :gw],
                                         mybir.ActivationFunctionType.Sigmoid,
                                         scale=-1.0)
                    # broadcast weights across 64 partitions via outer product
                    wub_ps = ps_bc.tile([64, 512], f32, tag="pb",
                                        name=f"wub_{rep_l}_{ci}")
                    nc.tensor.matmul(wub_ps[:, :gw], lhsT=ones1[:],
                                     rhs=wu[:, :gw], start=True, stop=True)
                    wib_ps = ps_bc.tile([64, 512], f32, tag="pb",
                                        name=f"wib_{rep_l}_{ci}")
                    nc.tensor.matmul(wib_ps[:, :gw], lhsT=ones1[:],
                                     rhs=wi[:, :gw], start=True, stop=True)
                    common = bpool.tile([64, 512], bf16, tag="common",
                                        name=f"common_{rep_l}_{ci}")
                    tmpc = bpool.tile([64, 512], bf16, tag="tmpc",
                                      name=f"tmpc_{rep_l}_{ci}")
                    nc.vector.tensor_mul(common[:, :gw], um, wub_ps[:, :gw])
                    nc.vector.tensor_mul(tmpc[:, :gw], im, wib_ps[:, :gw])
                    nc.vector.tensor_add(common[:, :gw], common[:, :gw],
                                         tmpc[:, :gw])
                    dfu = bpool.tile([64, 512], bf16, tag="dfu",
                                     name=f"dfu_{rep_l}_{ci}")
                    dfi = bpool.tile([64, 512], bf16, tag="dfi",
                                     name=f"dfi_{rep_l}_{ci}")
                    nc.vector.tensor_sub(dfu[:, :gw], um, common[:, :gw])
                    nc.vector.tensor_sub(dfi[:, :gw], im, common[:, :gw])
                    # u2+i2 accumulated; (uw1+iw1)@gT folded into one matmul
                    o2_ps = ps_bc.tile([64, 512], f32, tag="pb",
                                       name=f"o2_{rep_l}_{ci}")
                    nc.tensor.matmul(o2_ps[:, :gw], lhsT=uw_s[l][:],
                                     rhs=dfu[:, :gw], start=True, stop=False)
                    nc.tensor.matmul(o2_ps[:, :gw], lhsT=iw_s[l][:],
                                     rhs=dfi[:, :gw], start=False, stop=False)
                    nc.tensor.matmul(o2_ps[:, :gw], lhsT=wsum_s[l][:],
                                     rhs=gT_s[:, sl], start=False, stop=True)
                    # msg = u2 + i2 + (user_b+item_b) + common
                    nc.vector.scalar_tensor_tensor(
                        msgNT[:, sl], o2_ps[:, :gw], bsum_s[l][:],
                        common[:, :gw],
                        op0=mybir.AluOpType.add, op1=mybir.AluOpType.add)
                    nc.vector.tensor_add(he_acc[:, sl], he_acc[:, sl],
                                         msgNT[:, sl])

                # ==== Phase C: transpose msg to [GPAD, 64] fp8 tiles ====
                # msgP is fp8 scaled by 1/MSC (e4m3 max ~240); phase D
                # rescales by MSC when draining PSUM.
                NGT = GPAD // 128
                msgP = bpool.tile([128, NGT * D], fp8, tag="msgP",
                                  bufs=2, name=f"msgP_{rep_l}")
                msgR = bpool.tile([128, NGT * D], fp8, tag="msgR",
                                  bufs=2, name=f"msgR_{rep_l}")
                for t in range(NGT):
                    tp_ps = ps_bc.tile([128, 64], bf16, tag="pb",
                                       name=f"tp_{rep_l}_{t}")
                    nc.tensor.transpose(tp_ps[:, :],
                                        msgNT[:, t * 128:(t + 1) * 128],
                                        identB[:64, :64])
                    # q = fp8(msg/MSC); r = fp8(msg/MSC - dequant(q)):
                    # same scale, so both passes accumulate in one PSUM bank
                    qsc = bpool.tile([128, 64], f32, tag="qsc", bufs=3,
                                     name=f"qsc_{rep_l}_{t}")
                    nc.vector.scalar_tensor_tensor(
                        qsc[:, :], tp_ps[:, :], 1.0 / MSCS[l], zsc[:, :D],
                        op0=mybir.AluOpType.mult,
                        op1=mybir.AluOpType.bypass)
                    nc.vector.tensor_copy(msgP[:, t * D:(t + 1) * D],
                                          qsc[:, :])
                    nc.vector.tensor_sub(msgR[:, t * D:(t + 1) * D],
                                         qsc[:, :],
                                         msgP[:, t * D:(t + 1) * D])

                # ==== Phase D: node_out = full_hyper @ msg, transposed ====
                # fh loads span TWO 512-node chunks ([128,1024] per g-tile):
                # halves the DMA count and lifts each transfer above the
                # per-DMA descriptor-generation floor. The two chunks
                # accumulate in separate PSUM banks concurrently.
                NCH = 512
                for part, (fhT, nn_total, emb_res, ui_res, acc_res,
                           fout) in enumerate([
                        (fhTu, UC, ue_res, ui_u, acc_u, final_u),
                        (fhTi, IC, ie_res, ui_i, acc_i, final_i)]):
                    for n0 in range(0, nn_total, 2 * NCH):
                        nw2 = min(2 * NCH, nn_total - n0)
                        nwA = min(NCH, nw2)
                        nwB = nw2 - nwA
                        psA = ps_d.tile([64, NCH], f32, tag="pd",
                                        name=f"pdA_{rep_l}_{part}_{n0}")
                        if nwB:
                            psB = ps_d.tile([64, NCH], f32, tag="pd",
                                            name=f"pdB_{rep_l}_{part}_{n0}")
                        npair = NGT // 2
                        for t2 in range(npair):
                            g0 = t2 * 256
                            ft = dpool.tile([128, 2, 2 * NCH], fp8, tag="fh",
                                            name=f"fh_{rep_l}_{part}_{n0}_{t2}")
                            nc.scalar.dma_start(ft[:, 0, :nw2],
                                                fhT[g0:g0 + 128, n0:n0 + nw2])
                            nc.scalar.dma_start(ft[:, 1, :nw2],
                                                fhT[g0 + 128:g0 + 256,
                                                    n0:n0 + nw2])
                            for pi, mP in enumerate((msgP, msgR)):
                                lh = mP[:, 2 * t2 * D:(2 * t2 + 2) * D
                                        ].rearrange(
                                    "k (two m) -> k two m", two=2)
                                first = (t2 == 0 and pi == 0)
                                last = (t2 == npair - 1 and pi == 1)
                                nc.tensor.matmul(
                                    psA[:, :nwA], lhsT=lh,
                                    rhs=ft[:, :, :nwA],
                                    perf_mode=mybir.MatmulPerfMode.DoubleRow,
                                    start=first, stop=last)
                                if nwB:
                                    nc.tensor.matmul(
                                        psB[:, :nwB], lhsT=lh,
                                        rhs=ft[:, :, NCH:NCH + nwB],
                                        perf_mode=mybir.MatmulPerfMode.DoubleRow,
                                        start=first, stop=last)
                        for half, (ps, hn0, hnw) in enumerate(
                                [(psA, n0, nwA)] +
                                ([(psB, n0 + NCH, nwB)] if nwB else [])):
                            uiT = bpool.tile([64, NCH], bf16, tag="uiT",
                                             bufs=4,
                                             name=f"uiT_{rep_l}_{part}_{hn0}")
                            nc.vector.scalar_tensor_tensor(
                                uiT[:, :hnw], ps[:, :hnw], float(MSCS[l]),
                                zsc[:64, :hnw], op0=mybir.AluOpType.mult,
                                op1=mybir.AluOpType.bypass)
                            for s, (s0, ss) in enumerate(_ktiles(hnw)):
                                tp = ps_bc.tile([128, 64], bf16, tag="pb",
                                                name=f"dtp_{rep_l}_{part}_{hn0}_{s}")
                                nc.tensor.transpose(tp[:ss, :],
                                                    uiT[:, s0:s0 + ss],
                                                    identB[:64, :64])
                                ti = (hn0 + s0) // 128
                                tsl = slice(ti * D, (ti + 1) * D)
                                if l == 0:
                                    nc.vector.tensor_copy(ui_res[:ss, tsl],
                                                          tp[:ss, :])
                                    nc.vector.tensor_add(acc_res[:ss, tsl],
                                                         emb_res[:ss, tsl],
                                                         tp[:ss, :])
                                else:
                                    fo = opool.tile([128, D], f32, tag="fo",
                                                    name=f"fo_{rep_l}_{part}_{hn0}_{s}")
                                    nc.vector.tensor_add(fo[:ss, :],
                                                         acc_res[:ss, tsl],
                                                         tp[:ss, :])
                                    nc.sync.dma_start(
                                        fout[hn0 + s0:hn0 + s0 + ss, :],
                                        fo[:ss, :])

            # ==== final_he = group_emb + msg1 + msg2, transpose out ====
            for t, (g0, gg) in enumerate(GTL):
                tp_ps = ps_bc.tile([128, 64], f32, tag="pb", name=f"he_t_{t}")
                nc.tensor.transpose(tp_ps[:gg, :], he_acc[:, g0:g0 + gg],
                                    identF[:, :])
                ho = opool.tile([128, D], f32, tag="ho", name=f"ho_{t}")
                nc.vector.tensor_copy(ho[:gg, :], tp_ps[:gg, :])
                nc.scalar.dma_start(final_he[g0:g0 + gg, :], ho[:gg, :])

    nc.compile()
    return nc


_NC_CACHE = {}


def _get_nc():
    key = ("nc", os.environ.get("KREPS", "1"))
    if key not in _NC_CACHE:
        _NC_CACHE[key] = _build()
    return _NC_CACHE[key]


def make_in_maps(user_emb, item_emb, group_emb, user_hyper, item_hyper,
                 full_hyper, qc_w1, qc_b1, qc_w2, user_w, user_b, item_w,
                 item_b):
    import ml_dtypes
    bf = ml_dtypes.bfloat16
    f = np.float32
    f8 = ml_dtypes.float8_e4m3
    uh = np.asarray(user_hyper, f).astype(f8)
    ih = np.asarray(item_hyper, f).astype(f8)
    fh = np.asarray(full_hyper, f).astype(f8)
    ue = np.asarray(user_emb, f).astype(bf)
    ie = np.asarray(item_emb, f).astype(bf)
    rep = {
        "gT": np.ascontiguousarray(np.asarray(group_emb, f).T).astype(bf),
        "qc_w1": np.asarray(qc_w1, f).astype(bf),
        "qc_b1": np.asarray(qc_b1, f),
        "qc_w2": np.asarray(qc_w2, f).astype(bf),
        "user_w": np.asarray(user_w, f).astype(bf),
        "user_b": np.asarray(user_b, f),
        "item_w": np.asarray(item_w, f).astype(bf),
        "item_b": np.asarray(item_b, f),
    }
    in_maps = []
    for c in range(NCORES):
        us = slice(c * UC, (c + 1) * UC)
        isl = slice(c * IC, (c + 1) * IC)
        m = dict(rep)
        m["ue"] = np.ascontiguousarray(ue[us])
        m["ie"] = np.ascontiguousarray(ie[isl])
        m["uhT"] = np.ascontiguousarray(uh[:, us].T)
        m["ihT"] = np.ascontiguousarray(ih[:, isl].T)
        fu = np.zeros((GPAD, UC), f8)
        fu[:G] = fh[us, :].T
        m["fhTu"] = fu
        fi = np.zeros((GPAD, IC), f8)
        fi[:G] = fh[U + c * IC:U + (c + 1) * IC, :].T
        m["fhTi"] = fi
        in_maps.append(m)
    return in_maps


def assemble(results):
    out = np.empty((U + I + G, D), np.float32)
    for c in range(NCORES):
        out[c * UC:(c + 1) * UC] = results[c]["final_u"]
        out[U + c * IC:U + (c + 1) * IC] = results[c]["final_i"]
    out[U + I:] = results[0]["final_he"]
    return out


def kernel(user_emb, item_emb, group_emb, user_hyper, item_hyper, full_hyper,
           qc_w1, qc_b1, qc_w2, user_w, user_b, item_w, item_b,
           num_users=U, num_items=I):
    from concourse.bass_utils import run_bass_kernel_spmd
    nc = _get_nc()
    in_maps = make_in_maps(user_emb, item_emb, group_emb, user_hyper,
                           item_hyper, full_hyper, qc_w1, qc_b1, qc_w2,
                           user_w, user_b, item_w, item_b)
    res = run_bass_kernel_spmd(nc, in_maps, list(range(NCORES)))
    return assemble(res.results)



# revision 17
# speedup vs baseline: 1.0458x; 1.0458x over previous
"""HGNN encoder (2-layer hypergraph message passing) as an 8-core TRN2 Bass kernel.

Strategy (1D node partition, K-sharded G-matmuls, all-fp8 PE data path):
  - Each core owns a contiguous shard of user nodes (U/8) and item nodes
    (I/8), padded to multiples of 256 so every phase-A matmul is a full
    fp8 DoubleRow pair (K=256 per PE pass).
  - All fp8 quantization is MEAN-CENTERED with an exact rank-1 correction
    (rowsum(incidence) x column-mean, one K=2 bf16 matmul per PSUM tile):
    the all-positive incidence matrices amplify the common-mode of
    quantization noise ~1000x coherently, so the mean (and, for the host
    side, the exact column-mean of the quantization error) must bypass
    fp8. Rowsums of the fp8 incidence shards are host-precomputed and
    layer-invariant; column means are hi+lo bf16 pairs.
  - Phase A: incidence pairs stream as single [128, 2, G] DMAs; fp8
    DoubleRow matmuls (K=256) against fp8 embeddings (host-quantized for
    layer 1, device-quantized ui1/EU2 for layer 2) -> partial [64, G]
    messages in PSUM; bf16 AllReduce in two G-halves per part (user AR
    overlaps item matmuls).
  - Phase B (attention/update, tiny) runs redundantly per core in a
    transposed [64, G] layout.
  - Phase D: full_hyper@msg with TRANSPOSED output [64, nodes]:
    single-pass centered-fp8 msg, 8 DoubleRow G-pair matmuls per 512-node
    chunk from packed [128, 2, 1024] incidence loads, two PSUM banks in
    flight, rank-1 mean correction, then PE transposes back to node-major.
  - Queue plan (engine FIFOs block on the head's semaphore, so bulk
    buffer-gated streams get a dedicated queue): sync = incidence streams
    (rt pairs + fh tiles), scalar = activations + AR staging (right after
    the pdrain producer) + outputs, gpsimd = AR triggers only, vector =
    DVE compute + AR unstages.
KREPS (env) repeats the body for device-bound timing; kernel() always
uses the single-rep build.
"""

import os
import numpy as np

U, I, G, D = 30000, 60000, 2000, 64
L = 2

NCORES = 8
UC, IC = U // NCORES, I // NCORES
UCP = (UC + 255) // 256 * 256   # 3840: phase-A pair padding
ICP = (IC + 255) // 256 * 256   # 7680
GPAD = 2048


def _ktiles(n, step=128):
    return [(s, min(step, n - s)) for s in range(0, n, step)]


MSCS = (32.0, float(2 ** 27))  # per-layer msgP fp8 scale: |msg| maxes 887, 1.26e10
EU2 = 2048.0                   # layer-2 node-emb fp8 scale: |ui1| maxes 4.9e5
GCH = _ktiles(G, 512)     # 4 free-dim chunks for matmul N<=512
GTL = _ktiles(G, 128)     # 16 partition tiles


def _build():
    import concourse.bacc as bacc
    import concourse.mybir as mybir
    import concourse.tile as tile
    from concourse import masks

    reps = int(os.environ.get("KREPS", "1"))
    f32 = mybir.dt.float32
    bf16 = mybir.dt.bfloat16
    fp8 = mybir.dt.float8e4
    DR = mybir.MatmulPerfMode.DoubleRow
    AF = mybir.ActivationFunctionType
    nc = bacc.Bacc("TRN2", target_bir_lowering=False, debug=False,
                   num_devices=NCORES)

    def din(name, shape, dt=bf16):
        return nc.dram_tensor(name, shape, dt, kind="ExternalInput").ap()

    def dout(name, shape):
        return nc.dram_tensor(name, shape, mybir.dt.float32,
                              kind="ExternalOutput").ap()

    NTU, NTI = UCP // 128, ICP // 128    # 30 / 60 emb tiles
    NPU, NPI = NTU // 2, NTI // 2        # 15 / 30 DoubleRow pairs
    NGT = GPAD // 128                    # 16 msg tiles / 8 pairs

    ue = din("ue", [UCP, D])
    ie = din("ie", [ICP, D])
    uef8 = din("uef8", [UCP, D], fp8)
    ief8 = din("ief8", [ICP, D], fp8)
    uhTp = din("uhTp", [NPU * 128, 2, G], fp8)    # pair-packed incidence
    ihTp = din("ihTp", [NPI * 128, 2, G], fp8)
    fhTup = din("fhTup", [(NGT // 2) * 128, 2, UC], fp8)
    fhTip = din("fhTip", [(NGT // 2) * 128, 2, IC], fp8)
    mu_ue = din("mu_ue", [2, D])     # hi/lo emb col-means (+quant-err mean)
    mu_ie = din("mu_ie", [2, D])
    rsu2 = din("rsu2", [2, G])       # duplicated rowsums of fp8 uhT shard
    rsi2 = din("rsi2", [2, G])
    rsfu2 = din("rsfu2", [2, UC])    # duplicated rowsums of fp8 fhT cols
    rsfi2 = din("rsfi2", [2, IC])
    gT = din("gT", [D, G])
    qc_w1 = din("qc_w1", [L, D, D])
    qc_b1 = din("qc_b1", [L, D], f32)
    qc_w2 = din("qc_w2", [L, D, 1])
    user_w = din("user_w", [L, 2 * D, D])
    user_b = din("user_b", [L, D], f32)
    item_w = din("item_w", [L, 2 * D, D])
    item_b = din("item_b", [L, D], f32)

    final_u = dout("final_u", [UC, D])
    final_i = dout("final_i", [IC, D])
    final_he = dout("final_he", [G, D])
    kdbg = os.environ.get("KDBG") == "1"
    if kdbg:
        def dbgout(name, shape):
            return nc.dram_tensor(name, shape, mybir.dt.bfloat16,
                                  kind="ExternalOutput").ap()
        dbg_pd = dbgout("dbg_pd", [64, G])
        dbg_msgT = dbgout("dbg_msgT", [64, G])
        dbg_msgNT = dbgout("dbg_msgNT", [64, G])
        dbg_uiT = dbgout("dbg_uiT", [64, 512])
        dbg_muD2 = dbgout("dbg_muD2", [2, D])
        dbg_qmsg = dbgout("dbg_qmsg", [64, GPAD])
        dbg_msgP = nc.dram_tensor("dbg_msgP", [128, (GPAD // 128) * D],
                                  mybir.dt.float8e4,
                                  kind="ExternalOutput").ap()

    with tile.TileContext(nc) as tc:
        with (
            tc.tile_pool(name="const", bufs=1) as cpool,
            tc.tile_pool(name="resid", bufs=1) as rpool,
            tc.tile_pool(name="rhsA", bufs=4) as apool,
            tc.tile_pool(name="fhD", bufs=18) as dpool,
            tc.tile_pool(name="pb", bufs=2) as bpool,
            tc.tile_pool(name="outp", bufs=4) as opool,
            tc.tile_pool(name="psA", bufs=1, space="PSUM") as ps_a,
            tc.tile_pool(name="psD", bufs=2, space="PSUM") as ps_d,
            tc.tile_pool(name="psB", bufs=2, space="PSUM") as ps_bc,
            tc.tile_pool(name="dram", bufs=1, space="DRAM") as drpool,
        ):
            # ---- constants / weights -------------------------------------
            identB = cpool.tile([128, 128], bf16, tag="identB", name="identB")
            masks.make_identity(nc, identB[:])
            identF = cpool.tile([64, 64], f32, tag="identF", name="identF")
            masks.make_identity(nc, identF[:])
            ones1 = cpool.tile([1, D], bf16, tag="ones1", name="ones1")
            nc.vector.memset(ones1[:], 1.0)
            zsc = cpool.tile([128, 512], bf16, tag="zsc", name="zsc")
            nc.vector.memset(zsc[:], 0.0)

            gT_s = cpool.tile([D, GPAD], bf16, tag="gTs", name="gT_s")
            nc.scalar.dma_start(gT_s[:, :G], gT[:, :])
            mue_s = cpool.tile([2, D], bf16, tag="mue", name="mue_s")
            nc.scalar.dma_start(mue_s[:], mu_ue[:, :])
            mie_s = cpool.tile([2, D], bf16, tag="mie", name="mie_s")
            nc.scalar.dma_start(mie_s[:], mu_ie[:, :])
            rsu_s = cpool.tile([2, G], bf16, tag="rsu", name="rsu_s")
            nc.scalar.dma_start(rsu_s[:], rsu2[:, :])
            rsi_s = cpool.tile([2, G], bf16, tag="rsi", name="rsi_s")
            nc.scalar.dma_start(rsi_s[:], rsi2[:, :])
            rsfu_s = cpool.tile([2, UC], bf16, tag="rsfu", name="rsfu_s")
            nc.scalar.dma_start(rsfu_s[:], rsfu2[:, :])
            rsfi_s = cpool.tile([2, IC], bf16, tag="rsfi", name="rsfi_s")
            nc.scalar.dma_start(rsfi_s[:], rsfi2[:, :])

            w1_s, w2_s, b1_s, uw_s, iw_s, wsum_s, bsum_s = \
                [], [], [], [], [], [], []
            for l in range(L):
                w1 = cpool.tile([D, D], bf16, tag=f"w1_{l}", name=f"w1_{l}")
                nc.scalar.dma_start(w1[:], qc_w1[l])
                w1_s.append(w1)
                w2 = cpool.tile([D, 1], bf16, tag=f"w2_{l}", name=f"w2_{l}")
                nc.scalar.dma_start(w2[:], qc_w2[l])
                w2n = cpool.tile([D, 1], bf16, tag=f"w2n_{l}", name=f"w2n_{l}")
                nc.scalar.mul(w2n[:], w2[:], -1.0)
                w2_s.append((w2, w2n))
                b1 = cpool.tile([D, 1], f32, tag=f"b1_{l}", name=f"b1_{l}")
                nc.scalar.dma_start(b1[:], qc_b1[l].unsqueeze(1))
                b1_s.append(b1)
                uw0 = cpool.tile([D, D], bf16, tag=f"uw0_{l}", name=f"uw0_{l}")
                nc.scalar.dma_start(uw0[:], user_w[l][0:D, :])
                uw1 = cpool.tile([D, D], bf16, tag=f"uw1_{l}", name=f"uw1_{l}")
                nc.scalar.dma_start(uw1[:], user_w[l][D:2 * D, :])
                uw_s.append(uw0)
                iw0 = cpool.tile([D, D], bf16, tag=f"iw0_{l}", name=f"iw0_{l}")
                nc.scalar.dma_start(iw0[:], item_w[l][0:D, :])
                iw1 = cpool.tile([D, D], bf16, tag=f"iw1_{l}", name=f"iw1_{l}")
                nc.scalar.dma_start(iw1[:], item_w[l][D:2 * D, :])
                iw_s.append(iw0)
                ws = cpool.tile([D, D], bf16, tag=f"ws_{l}", name=f"ws_{l}")
                nc.vector.tensor_add(ws[:], uw1[:], iw1[:])
                wsum_s.append(ws)
                ub = cpool.tile([D, 1], f32, tag=f"ub_{l}", name=f"ub_{l}")
                nc.scalar.dma_start(ub[:], user_b[l].unsqueeze(1))
                ib = cpool.tile([D, 1], f32, tag=f"ib_{l}", name=f"ib_{l}")
                nc.scalar.dma_start(ib[:], item_b[l].unsqueeze(1))
                bs = cpool.tile([D, 1], f32, tag=f"bs_{l}", name=f"bs_{l}")
                nc.vector.tensor_add(bs[:], ub[:], ib[:])
                bsum_s.append(bs)

            # ---- residents ----------------------------------------------
            ue_res = rpool.tile([128, NTU * D], bf16, tag="ue_res",
                                name="ue_res")
            ie_res = rpool.tile([128, NTI * D], bf16, tag="ie_res",
                                name="ie_res")
            uef8_res = rpool.tile([128, NTU * D], fp8, tag="uef8_res",
                                  name="uef8_res")
            ief8_res = rpool.tile([128, NTI * D], fp8, tag="ief8_res",
                                  name="ief8_res")
            uif8_u = rpool.tile([128, NTU * D], fp8, tag="uif8_u",
                                name="uif8_u")
            uif8_i = rpool.tile([128, NTI * D], fp8, tag="uif8_i",
                                name="uif8_i")
            uiT_u = rpool.tile([D, UC], bf16, tag="uiT_u", name="uiT_u")
            uiT_i = rpool.tile([D, IC], bf16, tag="uiT_i", name="uiT_i")
            acc_u = rpool.tile([128, NTU * D], bf16, tag="acc_u",
                               name="acc_u")
            acc_i = rpool.tile([128, NTI * D], bf16, tag="acc_i",
                               name="acc_i")
            he_acc = rpool.tile([D, GPAD], f32, tag="he_acc", name="he_acc")
            # zero the fp8 layer-2 emb residents once: the node-pad rows
            # beyond UC/IC must multiply as 0 in phase A (never written by
            # the phase-D drain, and fp8 garbage can be NaN).
            nc.vector.memset(uif8_u[:], 0.0)
            nc.vector.memset(uif8_i[:], 0.0)
            for t in range(NTU):
                nc.sync.dma_start(ue_res[:, t * D:(t + 1) * D],
                                  ue[t * 128:(t + 1) * 128, :])
                nc.sync.dma_start(uef8_res[:, t * D:(t + 1) * D],
                                  uef8[t * 128:(t + 1) * 128, :])
            for t in range(NTI):
                nc.scalar.dma_start(ie_res[:, t * D:(t + 1) * D],
                                    ie[t * 128:(t + 1) * 128, :])
                nc.scalar.dma_start(ief8_res[:, t * D:(t + 1) * D],
                                    ief8[t * 128:(t + 1) * 128, :])
            nc.vector.tensor_copy(he_acc[:, :G], gT_s[:, :G])

            muA2 = [None, None]   # device [2,64] scaled ui1 col-means

            for rep_l in range(reps * L):
                rep, l = divmod(rep_l, L)
                # ==== Phase A: partial messages, K-sharded over nodes ====
                skip_cc = os.environ.get("KSKIP_CC") == "1"
                msgTs = []
                for part, (hyTp, npair, emb0, rs_s, mu_host) in enumerate([
                        (uhTp, NPU, uef8_res, rsu_s, mue_s),
                        (ihTp, NPI, ief8_res, rsi_s, mie_s)]):
                    if l == 0:
                        lhs_res, mu2 = emb0, mu_host
                    else:
                        lhs_res = uif8_u if part == 0 else uif8_i
                        mu2 = muA2[part]
                    ps_msg = ps_a.tile([64, GPAD], f32, tag="msgps",
                                       name=f"msgps_{rep_l}_{part}")
                    for p in range(npair):
                        rt = apool.tile([128, 2, G], fp8, tag="rhsA",
                                        name=f"rhsA_{rep_l}_{part}_{p}")
                        nc.sync.dma_start(rt[:, :, :],
                                          hyTp[p * 128:(p + 1) * 128, :, :])
                        lh = lhs_res[:, 2 * p * D:(2 * p + 2) * D].rearrange(
                            "k (two m) -> k two m", two=2)
                        for (g0, gw) in GCH:
                            nc.tensor.matmul(
                                ps_msg[:, g0:g0 + gw],
                                lhsT=lh, rhs=rt[:, :, g0:g0 + gw],
                                perf_mode=DR,
                                start=(p == 0), stop=False)
                    # exact rank-1 mean correction: += mu (x) rowsum
                    # (closes each chunk's PSUM accumulation group)
                    for (g0, gw) in GCH:
                        nc.tensor.matmul(ps_msg[:, g0:g0 + gw],
                                         lhsT=mu2[:, :],
                                         rhs=rs_s[:, g0:g0 + gw],
                                         start=False, stop=True)
                    pdrain = bpool.tile([64, G], bf16, tag=f"pdrain{part}",
                                        bufs=1, name=f"pdrain_{rep_l}_{part}")
                    nc.scalar.activation(pdrain[:, :], ps_msg[:, :G],
                                         AF.Copy,
                                         scale=(1.0 if l == 0 else EU2))
                    # AllReduce in two G-halves. Stage + trigger here (right
                    # after the pdrain producer on scalar / gpsimd); the
                    # unstages are emitted after BOTH parts so a pending AR
                    # never head-of-line-blocks the other part's stages.
                    msgT = bpool.tile([64, GPAD], bf16, tag=f"msgT{part}",
                                      bufs=1, name=f"msgT_{rep_l}_{part}")
                    halves = []
                    for hf, (h0, hw) in enumerate(((0, 1024), (1024, G - 1024))):
                        cc_in = drpool.tile(
                            [64, hw], bf16, tag=f"cc_in_{rep_l}_{part}_{hf}",
                            name=f"cc_in_{rep_l}_{part}_{hf}")
                        nc.scalar.dma_start(cc_in[:, :],
                                            pdrain[:, h0:h0 + hw])
                        if skip_cc:
                            cc_out = cc_in
                        else:
                            cc_out = drpool.tile(
                                [64, hw], bf16,
                                tag=f"cc_out_{rep_l}_{part}_{hf}",
                                addr_space="Shared",
                                name=f"cc_out_{rep_l}_{part}_{hf}")
                            nc.gpsimd.collective_compute(
                                "AllReduce", mybir.AluOpType.add,
                                ins=[cc_in.opt()], outs=[cc_out.opt()],
                                replica_groups=[list(range(NCORES))])
                        halves.append((h0, hw, cc_out))
                    msgTs.append((msgT, halves))
                    if kdbg and rep_l == 0 and part == 0:
                        nc.scalar.dma_start(dbg_pd[:, :], pdrain[:, :])
                for msgT, halves in msgTs:
                    for h0, hw, cc_out in halves:
                        nc.scalar.dma_start(msgT[:, h0:h0 + hw], cc_out[:, :])
                msgT_u, msgT_i = msgTs[0][0], msgTs[1][0]
                if kdbg and rep_l == 0:
                    nc.scalar.dma_start(dbg_msgT[:, :], msgT_u[:, :G])

                # ==== Phase B: attention + node-update weights (full G) ====
                msgNT = bpool.tile([64, GPAD], bf16, tag="msgNT", bufs=1,
                                   name=f"msgNT_{rep_l}")
                nc.vector.memset(msgNT[:, G:GPAD], 0.0)
                for ci, (g0, gw) in enumerate(GCH):
                    sl = slice(g0, g0 + gw)
                    um = msgT_u[:, sl]
                    im = msgT_i[:, sl]
                    # h = tanh(msg @ W1 + b1), transposed layout
                    hu_ps = ps_bc.tile([64, 512], f32, tag="pb",
                                       name=f"hu_{rep_l}_{ci}")
                    nc.tensor.matmul(hu_ps[:, :gw], lhsT=w1_s[l][:], rhs=um,
                                     start=True, stop=True)
                    hu = bpool.tile([64, 512], bf16, tag="hu",
                                    name=f"hus_{rep_l}_{ci}")
                    nc.scalar.activation(hu[:, :gw], hu_ps[:, :gw],
                                         AF.Tanh, bias=b1_s[l][:])
                    hi_ps = ps_bc.tile([64, 512], f32, tag="pb",
                                       name=f"hi_{rep_l}_{ci}")
                    nc.tensor.matmul(hi_ps[:, :gw], lhsT=w1_s[l][:], rhs=im,
                                     start=True, stop=True)
                    hi = bpool.tile([64, 512], bf16, tag="hi",
                                    name=f"his_{rep_l}_{ci}")
                    nc.scalar.activation(hi[:, :gw], hi_ps[:, :gw],
                                         AF.Tanh, bias=b1_s[l][:])
                    # attention logit diff a_u - a_i in one bank;
                    # softmax over 2 == sigmoid(+-diff)
                    ad_ps = ps_bc.tile([1, 512], f32, tag="pb",
                                       name=f"ad_{rep_l}_{ci}")
                    nc.tensor.matmul(ad_ps[:, :gw], lhsT=w2_s[l][0][:],
                                     rhs=hu[:, :gw], start=True, stop=False)
                    nc.tensor.matmul(ad_ps[:, :gw], lhsT=w2_s[l][1][:],
                                     rhs=hi[:, :gw], start=False, stop=True)
                    wu = bpool.tile([1, 512], bf16, tag="wud", bufs=4,
                                    name=f"wu_{rep_l}_{ci}")
                    nc.scalar.activation(wu[:, :gw], ad_ps[:, :gw],
                                         AF.Sigmoid)
                    wi = bpool.tile([1, 512], bf16, tag="wud", bufs=4,
                                    name=f"wi_{rep_l}_{ci}")
                    nc.scalar.activation(wi[:, :gw], ad_ps[:, :gw],
                                         AF.Sigmoid, scale=-1.0)
                    # broadcast weights across 64 partitions via outer product
                    wub_ps = ps_bc.tile([64, 512], f32, tag="pb",
                                        name=f"wub_{rep_l}_{ci}")
                    nc.tensor.matmul(wub_ps[:, :gw], lhsT=ones1[:],
                                     rhs=wu[:, :gw], start=True, stop=True)
                    wib_ps = ps_bc.tile([64, 512], f32, tag="pb",
                                        name=f"wib_{rep_l}_{ci}")
                    nc.tensor.matmul(wib_ps[:, :gw], lhsT=ones1[:],
                                     rhs=wi[:, :gw], start=True, stop=True)
                    common = bpool.tile([64, 512], bf16, tag="common",
                                        name=f"common_{rep_l}_{ci}")
                    tmpc = bpool.tile([64, 512], bf16, tag="tmpc",
                                      name=f"tmpc_{rep_l}_{ci}")
                    nc.vector.tensor_mul(common[:, :gw], um, wub_ps[:, :gw])
                    nc.vector.tensor_mul(tmpc[:, :gw], im, wib_ps[:, :gw])
                    nc.vector.tensor_add(common[:, :gw], common[:, :gw],
                                         tmpc[:, :gw])
                    dfu = bpool.tile([64, 512], bf16, tag="dfu",
                                     name=f"dfu_{rep_l}_{ci}")
                    dfi = bpool.tile([64, 512], bf16, tag="dfi",
                                     name=f"dfi_{rep_l}_{ci}")
                    nc.vector.tensor_sub(dfu[:, :gw], um, common[:, :gw])
                    nc.vector.tensor_sub(dfi[:, :gw], im, common[:, :gw])
                    # u2+i2 accumulated; (uw1+iw1)@gT folded into one matmul
                    o2_ps = ps_bc.tile([64, 512], f32, tag="pb",
                                       name=f"o2_{rep_l}_{ci}")
                    nc.tensor.matmul(o2_ps[:, :gw], lhsT=uw_s[l][:],
                                     rhs=dfu[:, :gw], start=True, stop=False)
                    nc.tensor.matmul(o2_ps[:, :gw], lhsT=iw_s[l][:],
                                     rhs=dfi[:, :gw], start=False, stop=False)
                    nc.tensor.matmul(o2_ps[:, :gw], lhsT=wsum_s[l][:],
                                     rhs=gT_s[:, sl], start=False, stop=True)
                    # msg = u2 + i2 + (user_b+item_b) + common
                    nc.vector.scalar_tensor_tensor(
                        msgNT[:, sl], o2_ps[:, :gw], bsum_s[l][:],
                        common[:, :gw],
                        op0=mybir.AluOpType.add, op1=mybir.AluOpType.add)
                    nc.vector.tensor_add(he_acc[:, sl], he_acc[:, sl],
                                         msgNT[:, sl])

                if kdbg and rep_l == 0:
                    nc.scalar.dma_start(dbg_msgNT[:, :], msgNT[:, :G])
                # ==== Phase C: center + quantize msg, transpose to fp8 ====
                # mu_msg = row-mean of msgNT; q = (msg - mu)/MSC in bf16;
                # msgP = fp8(q) tiles [128, 64]; muD2 = hi/lo bf16 [2, 64]
                # of mu/MSC for phase D's rank-1 correction.
                msum = bpool.tile([D, 1], f32, tag="msum", name=f"msum_{rep_l}")
                nc.vector.reduce_sum(msum[:, :], msgNT[:, :G],
                                     axis=mybir.AxisListType.X)
                vmu = bpool.tile([D, 1], f32, tag="vmu", name=f"vmu_{rep_l}")
                nc.scalar.mul(vmu[:, :], msum[:, :], 1.0 / (G * MSCS[l]))
                nbm = bpool.tile([D, 1], f32, tag="nbm", name=f"nbm_{rep_l}")
                nc.scalar.mul(nbm[:, :], msum[:, :], -1.0 / G)
                muhl = bpool.tile([D, 2], bf16, tag="muhl",
                                  name=f"muhl_{rep_l}")
                hif = bpool.tile([D, 1], f32, tag="hif", name=f"hif_{rep_l}")
                nc.vector.tensor_copy(muhl[:, 0:1], vmu[:, :])
                nc.vector.tensor_copy(hif[:, :], muhl[:, 0:1])
                rem = bpool.tile([D, 1], f32, tag="rem", name=f"rem_{rep_l}")
                nc.vector.tensor_sub(rem[:, :], vmu[:, :], hif[:, :])
                nc.vector.tensor_copy(muhl[:, 1:2], rem[:, :])
                mu2_ps = ps_bc.tile([2, D], bf16, tag="pb",
                                    name=f"mu2ps_{rep_l}")
                nc.tensor.transpose(mu2_ps[:, :], muhl[:, :],
                                    identB[:64, :64])
                muD2 = bpool.tile([2, D], bf16, tag="muD2",
                                  name=f"muD2_{rep_l}")
                nc.vector.tensor_copy(muD2[:, :], mu2_ps[:, :])
                if kdbg and rep_l == 0:
                    nc.scalar.dma_start(dbg_muD2[:, :], muD2[:, :])

                # qmsg = msg - mu (bf16, unscaled); the 1/MSC scale rides the
                # transpose's scaled identity
                qmsg = bpool.tile([64, GPAD], bf16, tag="qmsg", bufs=1,
                                  name=f"qmsg_{rep_l}")
                for (g0, gw) in _ktiles(GPAD, 512):
                    nc.vector.scalar_tensor_tensor(
                        qmsg[:, g0:g0 + gw], msgNT[:, g0:g0 + gw],
                        nbm[:, :], zsc[:64, :gw],
                        op0=mybir.AluOpType.add,
                        op1=mybir.AluOpType.bypass)
                msgP = bpool.tile([128, NGT * D], fp8, tag="msgP",
                                  bufs=2, name=f"msgP_{rep_l}")
                for t in range(NGT):
                    tp_ps = ps_bc.tile([128, 64], bf16, tag="pb",
                                       name=f"tp_{rep_l}_{t}")
                    nc.tensor.transpose(tp_ps[:, :],
                                        qmsg[:, t * 128:(t + 1) * 128],
                                        identB[:64, :64])
                    nc.vector.scalar_tensor_tensor(
                        msgP[:, t * D:(t + 1) * D], tp_ps[:, :],
                        1.0 / MSCS[l], zsc[:, :D],
                        op0=mybir.AluOpType.mult,
                        op1=mybir.AluOpType.bypass)

                if kdbg and rep_l == 0:
                    nc.scalar.dma_start(dbg_qmsg[:, :], qmsg[:, :])
                    nc.scalar.dma_start(dbg_msgP[:, :], msgP[:, :])
                # ==== Phase D: node_out = full_hyper @ msg, transposed ====
                NCH = 512
                for part, (fhTp3, rsf_s, nn_total, emb_res, uif8_res,
                           uiT_res, acc_res, fout) in enumerate([
                        (fhTup, rsfu_s, UC, ue_res, uif8_u, uiT_u, acc_u,
                         final_u),
                        (fhTip, rsfi_s, IC, ie_res, uif8_i, uiT_i, acc_i,
                         final_i)]):
                    for n0 in range(0, nn_total, 2 * NCH):
                        nw2 = min(2 * NCH, nn_total - n0)
                        nwA = min(NCH, nw2)
                        nwB = nw2 - nwA
                        psA = ps_d.tile([64, NCH], f32, tag="pd",
                                        name=f"pdA_{rep_l}_{part}_{n0}")
                        if nwB:
                            psB = ps_d.tile([64, NCH], f32, tag="pd",
                                            name=f"pdB_{rep_l}_{part}_{n0}")
                        npair = NGT // 2
                        for t2 in range(npair):
                            ft = dpool.tile([128, 2, 2 * NCH], fp8, tag="fh",
                                            name=f"fh_{rep_l}_{part}_{n0}_{t2}")
                            nc.sync.dma_start(
                                ft[:, :, :nw2],
                                fhTp3[t2 * 128:(t2 + 1) * 128, :,
                                      n0:n0 + nw2])
                            lh = msgP[:, 2 * t2 * D:(2 * t2 + 2) * D
                                      ].rearrange("k (two m) -> k two m",
                                                  two=2)
                            first = (t2 == 0)
                            nc.tensor.matmul(
                                psA[:, :nwA], lhsT=lh,
                                rhs=ft[:, :, :nwA],
                                perf_mode=DR,
                                start=first, stop=False)
                            if nwB:
                                nc.tensor.matmul(
                                    psB[:, :nwB], lhsT=lh,
                                    rhs=ft[:, :, NCH:NCH + nwB],
                                    perf_mode=DR,
                                    start=first, stop=False)
                        # rank-1 mean correction closes each PSUM group
                        nc.tensor.matmul(psA[:, :nwA], lhsT=muD2[:, :],
                                         rhs=rsf_s[:, n0:n0 + nwA],
                                         start=False, stop=True)
                        if nwB:
                            nc.tensor.matmul(
                                psB[:, :nwB], lhsT=muD2[:, :],
                                rhs=rsf_s[:, n0 + NCH:n0 + NCH + nwB],
                                start=False, stop=True)
                        for half, (ps, hn0, hnw) in enumerate(
                                [(psA, n0, nwA)] +
                                ([(psB, n0 + NCH, nwB)] if nwB else [])):
                            nc.vector.scalar_tensor_tensor(
                                uiT_res[:, hn0:hn0 + hnw], ps[:, :hnw],
                                float(MSCS[l]), zsc[:64, :hnw],
                                op0=mybir.AluOpType.mult,
                                op1=mybir.AluOpType.bypass)
                            for s, (s0, ss) in enumerate(_ktiles(hnw)):
                                tp = ps_bc.tile([128, 64], bf16, tag="pb",
                                                name=f"dtp_{rep_l}_{part}_{hn0}_{s}")
                                nc.tensor.transpose(
                                    tp[:ss, :],
                                    uiT_res[:, hn0 + s0:hn0 + s0 + ss],
                                    identB[:64, :64])
                                ti = (hn0 + s0) // 128
                                tsl = slice(ti * D, (ti + 1) * D)
                                if l == 0:
                                    nc.vector.tensor_add(acc_res[:ss, tsl],
                                                         emb_res[:ss, tsl],
                                                         tp[:ss, :])
                                else:
                                    fo = opool.tile([128, D], f32, tag="fo",
                                                    name=f"fo_{rep_l}_{part}_{hn0}_{s}")
                                    nc.vector.tensor_add(fo[:ss, :],
                                                         acc_res[:ss, tsl],
                                                         tp[:ss, :])
                                    nc.scalar.dma_start(
                                        fout[hn0 + s0:hn0 + s0 + ss, :],
                                        fo[:ss, :])
                    if kdbg and rep_l == 0 and part == 0:
                        nc.scalar.dma_start(dbg_uiT[:, :], uiT_res[:, :512])
                    if l == 0:
                        # ui1 col-mean (for centered fp8 quantize + layer-2
                        # rank-1 correction), then quantize ui1 -> uif8
                        usum = bpool.tile([D, 1], f32, tag="usum",
                                          name=f"usum_{rep_l}_{part}")
                        nc.vector.reduce_sum(usum[:, :], uiT_res[:, :],
                                             axis=mybir.AxisListType.X)
                        vu = bpool.tile([D, 1], f32, tag="vu",
                                        name=f"vu_{rep_l}_{part}")
                        nc.scalar.mul(vu[:, :], usum[:, :],
                                      1.0 / (nn_total * EU2))
                        nbu = bpool.tile([D, 1], f32, tag="nbu",
                                         name=f"nbu_{rep_l}_{part}")
                        nc.scalar.mul(nbu[:, :], usum[:, :], -1.0 / nn_total)
                        muhl2 = bpool.tile([D, 2], bf16, tag="muhl2",
                                           name=f"muhl2_{rep_l}_{part}")
                        hif2 = bpool.tile([D, 1], f32, tag="hif2",
                                          name=f"hif2_{rep_l}_{part}")
                        nc.vector.tensor_copy(muhl2[:, 0:1], vu[:, :])
                        nc.vector.tensor_copy(hif2[:, :], muhl2[:, 0:1])
                        rem2 = bpool.tile([D, 1], f32, tag="rem2",
                                          name=f"rem2_{rep_l}_{part}")
                        nc.vector.tensor_sub(rem2[:, :], vu[:, :], hif2[:, :])
                        nc.vector.tensor_copy(muhl2[:, 1:2], rem2[:, :])
                        mu2b_ps = ps_bc.tile([2, D], bf16, tag="pb",
                                             name=f"mu2b_{rep_l}_{part}")
                        nc.tensor.transpose(mu2b_ps[:, :], muhl2[:, :],
                                            identB[:64, :64])
                        mA = rpool.tile([2, D], bf16, tag=f"muA2_{part}",
                                        name=f"muA2_{part}_{rep}")
                        nc.vector.tensor_copy(mA[:, :], mu2b_ps[:, :])
                        muA2[part] = mA
                        for c0 in range(0, nn_total, NCH):
                            cw = min(NCH, nn_total - c0)
                            qc = bpool.tile([64, NCH], bf16, tag="qc",
                                            bufs=4,
                                            name=f"qc_{rep_l}_{part}_{c0}")
                            nc.vector.scalar_tensor_tensor(
                                qc[:, :cw], uiT_res[:, c0:c0 + cw],
                                nbu[:, :], zsc[:64, :cw],
                                op0=mybir.AluOpType.add,
                                op1=mybir.AluOpType.bypass)
                            for s0, ss in _ktiles(cw):
                                tq = ps_bc.tile([128, 64], bf16, tag="pb",
                                                name=f"tq_{rep_l}_{part}_{c0}_{s0}")
                                nc.tensor.transpose(tq[:ss, :],
                                                    qc[:, s0:s0 + ss],
                                                    identB[:64, :64])
                                ti = (c0 + s0) // 128
                                nc.vector.scalar_tensor_tensor(
                                    uif8_res[:ss, ti * D:(ti + 1) * D],
                                    tq[:ss, :], 1.0 / EU2, zsc[:ss, :D],
                                    op0=mybir.AluOpType.mult,
                                    op1=mybir.AluOpType.bypass)

            # ==== final_he = group_emb + msg1 + msg2, transpose out ====
            for t, (g0, gg) in enumerate(GTL):
                tp_ps = ps_bc.tile([128, 64], f32, tag="pb", name=f"he_t_{t}")
                nc.tensor.transpose(tp_ps[:gg, :], he_acc[:, g0:g0 + gg],
                                    identF[:, :])
                ho = opool.tile([128, D], f32, tag="ho", name=f"ho_{t}")
                nc.vector.tensor_copy(ho[:gg, :], tp_ps[:gg, :])
                nc.scalar.dma_start(final_he[g0:g0 + gg, :], ho[:gg, :])

    nc.compile()
    return nc


_NC_CACHE = {}


def _get_nc():
    key = ("nc", os.environ.get("KREPS", "1"))
    if key not in _NC_CACHE:
        _NC_CACHE[key] = _build()
    return _NC_CACHE[key]


def _hilo(v):
    import ml_dtypes
    bf = ml_dtypes.bfloat16
    hi = v.astype(bf)
    lo = (v - hi.astype(np.float32)).astype(bf)
    return np.stack([hi, lo]).astype(bf)


def make_in_maps(user_emb, item_emb, group_emb, user_hyper, item_hyper,
                 full_hyper, qc_w1, qc_b1, qc_w2, user_w, user_b, item_w,
                 item_b):
    import ml_dtypes
    bf = ml_dtypes.bfloat16
    f = np.float32
    f8 = ml_dtypes.float8_e4m3
    uh = np.asarray(user_hyper, f).astype(f8)
    ih = np.asarray(item_hyper, f).astype(f8)
    fh = np.asarray(full_hyper, f).astype(f8)
    ue = np.asarray(user_emb, f)
    ie = np.asarray(item_emb, f)
    NPU, NPI = UCP // 256, ICP // 256
    NGP = GPAD // 256
    rep = {
        "gT": np.ascontiguousarray(np.asarray(group_emb, f).T).astype(bf),
        "qc_w1": np.asarray(qc_w1, f).astype(bf),
        "qc_b1": np.asarray(qc_b1, f),
        "qc_w2": np.asarray(qc_w2, f).astype(bf),
        "user_w": np.asarray(user_w, f).astype(bf),
        "user_b": np.asarray(user_b, f),
        "item_w": np.asarray(item_w, f).astype(bf),
        "item_b": np.asarray(item_b, f),
    }

    def pack_pairs(a, npair):
        # [npair*256, W] -> [npair*128, 2, W]: row (p*128+r, j) = a[(2p+j)*128+r]
        W = a.shape[1]
        return np.ascontiguousarray(
            a.reshape(npair, 2, 128, W).transpose(0, 2, 1, 3)
        ).reshape(npair * 128, 2, W)

    in_maps = []
    for c in range(NCORES):
        us = slice(c * UC, (c + 1) * UC)
        isl = slice(c * IC, (c + 1) * IC)
        m = dict(rep)
        uep = np.zeros((UCP, D), f)
        uep[:UC] = ue[us]
        iep = np.zeros((ICP, D), f)
        iep[:IC] = ie[isl]
        m["ue"] = uep.astype(bf)
        m["ie"] = iep.astype(bf)
        # centered fp8 embeddings + col-means with the exact quantization
        # error mean folded in (kills the coherent common-mode)
        mu_u = uep[:UC].mean(0)
        mu_i = iep[:IC].mean(0)
        uq = np.zeros((UCP, D), f8)
        uq[:UC] = (uep[:UC] - mu_u).astype(f8)
        iq = np.zeros((ICP, D), f8)
        iq[:IC] = (iep[:IC] - mu_i).astype(f8)
        m["uef8"] = uq
        m["ief8"] = iq
        mu_u = mu_u + (uep[:UC] - mu_u - uq[:UC].astype(f)).mean(0)
        mu_i = mu_i + (iep[:IC] - mu_i - iq[:IC].astype(f)).mean(0)
        m["mu_ue"] = _hilo(mu_u)
        m["mu_ie"] = _hilo(mu_i)
        uhp = np.zeros((UCP, G), f8)
        uhp[:UC] = uh[:, us].T
        m["uhTp"] = pack_pairs(uhp, NPU)
        ihp = np.zeros((ICP, G), f8)
        ihp[:IC] = ih[:, isl].T
        m["ihTp"] = pack_pairs(ihp, NPI)
        m["rsu2"] = np.broadcast_to(
            uhp.astype(f).sum(0).astype(bf), (2, G)).copy()
        m["rsi2"] = np.broadcast_to(
            ihp.astype(f).sum(0).astype(bf), (2, G)).copy()
        fu = np.zeros((GPAD, UC), f8)
        fu[:G] = fh[us, :].T
        m["fhTup"] = pack_pairs(fu, NGP)
        fi = np.zeros((GPAD, IC), f8)
        fi[:G] = fh[U + c * IC:U + (c + 1) * IC, :].T
        m["fhTip"] = pack_pairs(fi, NGP)
        m["rsfu2"] = np.broadcast_to(
            fu.astype(f).sum(0).astype(bf), (2, UC)).copy()
        m["rsfi2"] = np.broadcast_to(
            fi.astype(f).sum(0).astype(bf), (2, IC)).copy()
        in_maps.append(m)
    return in_maps


def assemble(results):
    out = np.empty((U + I + G, D), np.float32)
    for c in range(NCORES):
        out[c * UC:(c + 1) * UC] = results[c]["final_u"]
        out[U + c * IC:U + (c + 1) * IC] = results[c]["final_i"]
    out[U + I:] = results[0]["final_he"]
    return out


def kernel(user_emb, item_emb, group_emb, user_hyper, item_hyper, full_hyper,
           qc_w1, qc_b1, qc_w2, user_w, user_b, item_w, item_b,
           num_users=U, num_items=I):
    from concourse.bass_utils import run_bass_kernel_spmd
    nc = _get_nc()
    in_maps = make_in_maps(user_emb, item_emb, group_emb, user_hyper,
                           item_hyper, full_hyper, qc_w1, qc_b1, qc_w2,
                           user_w, user_b, item_w, item_b)
    res = run_bass_kernel_spmd(nc, in_maps, list(range(NCORES)))
    return assemble(res.results)
